# revision 11
# baseline (speedup 1.0000x reference)
"""TRN2 Bass kernel for nn_Aggregator (GNN message passing + bi-interaction).

Computes, for graph with N=100000 nodes, E=800000 edges, D=128:
    msgs = entity_embed[src] * att                  (per-edge message)
    N_h  = segment_sum(msgs, dst)                   (scatter-add to nodes)
    out  = LRelu((node+N_h)@W1+b1) + LRelu((node*N_h)@W2+b2)

Strategy (8 NeuronCores, SPMD, no collectives):
  * Edges are bucketed by dst//12500 -> owning core; each core computes the
    full output rows for its 12500-node partition.
  * The host materializes per-edge messages (embed[src]*att, fp16) into a
    degree-sorted slotted layout -- the sharding hint's "messages" input.
    Nodes are sorted by in-degree ASCENDING (pads first) and renumbered so
    group processing order == memory order: group i covers final ranks
    [nboff[i], nboff[i]+gw), needs CC[i] = max in-group degree occurrence
    planes, and its messages live plane-major at
    col = msoff[i] + c*gw + rank_in_group.  The two biggest-CC groups are
    moved a few slots before the end so the pipeline tail is short.
  * Device segment-sum = binary halving tree of contiguous fp16 DVE
    tensor_tensor adds (~1.0ns/col; tensor_reduce measured 1.05 with no
    grouping freedom).  No gather (the old SWDGE dma_gather serialized
    ~386us of descriptor generation on GpSimd), no one-hot matmul.
  * msgs and the node table stream in ramped superblocks (0.5MB..4MB, ~8
    dma_starts each instead of 25/1) to amortize the ~2us fixed per-DMA
    completion latency and ride the 436 GB/s DMA setup-knee; msgs for the
    first superblock are issued before everything else so compute starts
    ~6us in.
  * x1 = node+N_h is never materialized: PE computes
    o1 = W1^T@nodeT + W1^T@N_hT by PSUM accumulation (fp16 matmuls, f32
    accumulate -- also kills one fp16 rounding).  x2 = nodeT*N_hT on
    GpSimd (its port never contends with DVE tensor_tensor), o2 deferred
    one group so the PE in-order queue never waits on GpSimd;
    bias+LeakyReLU on Scalar (f32 bias APs).  The final r1+r2 runs on PE
    as two accumulating identity matmuls (deferred two groups), Scalar
    copies PSUM->SBUF, fp16 outT stored per group.
  * The host inverse-permutes and upcasts at the end.  The CC schedule is
    shared across cores (SPMD single program), per-group max over cores
    (total slots ~ E/8 + ~6%).
"""
import sys

sys.path.insert(0, "/opt/trn_rl_repo")

import numpy as np

N_NODES = 100000
N_EDGES = 800000
D = 128
NCORES = 8
NPC = N_NODES // NCORES          # 12500 nodes per core
W = 128
NWIN = (NPC + W - 1) // W        # 98 windows per core
NPC_PAD = NWIN * W               # padded node count per core (12544)
NPAD = NPC_PAD - NPC             # 44 pad nodes (rank 0.., zero embed)
GRP = 4
GW = GRP * W                     # 512 node columns per group
NG = (NWIN + GRP - 1) // GRP     # 25 groups (one is 256 wide)
SB_RAMP = (2048, 4096, 8192)     # first superblock slot budgets
SB_CAP = 16384                   # steady-state superblock slots (32KB/part)
SB_GWCAP = 4096                  # max node columns per superblock

_BUILD_CACHE = {}


def _nat_widths():
    """Natural group widths: 24x512 + 1x256 (the top-degree group)."""
    return [GW] * (NG - 1) + [NPC_PAD - (NG - 1) * GW]


def _proc_perm():
    """Processing order of natural groups: ascending CC (= natural order)
    except groups 2,3 (small CC) are saved for last — the pipeline drain
    then trails a ~1-plane group instead of the max-CC one."""
    if NG < 6:
        return list(range(NG))
    return [0, 1] + list(range(4, NG)) + [2, 3]


def _schedule(cc_p):
    """Superblocks + offsets over processed positions.

    cc_p[i] = occurrence planes of processed group i.
    Returns (gw_p, nboff, msoff, superblocks as lists of positions).
    """
    widths = _nat_widths()
    perm = _proc_perm()
    gw_p = [widths[j] for j in perm]
    nboff = np.concatenate(([0], np.cumsum(gw_p))).astype(np.int64)
    slots = [int(cc_p[i]) * gw_p[i] for i in range(NG)]
    msoff = np.concatenate(([0], np.cumsum(slots))).astype(np.int64)
    sbs, cur, cur_slots, cur_gw, ramp = [], [], 0, 0, 0
    for i in range(NG):
        cap = SB_RAMP[ramp] if ramp < len(SB_RAMP) else SB_CAP
        if cur and (cur_slots + slots[i] > cap or cur_gw + gw_p[i] > SB_GWCAP):
            sbs.append(cur)
            cur, cur_slots, cur_gw = [], 0, 0
            ramp += 1
        cur.append(i)
        cur_slots += slots[i]
        cur_gw += gw_p[i]
    if cur:
        sbs.append(cur)
    return gw_p, nboff, msoff, sbs


def _build(cc_p):
    """Build + bacc-compile the SPMD Bass program for a CC schedule."""
    key = tuple(cc_p)
    if key in _BUILD_CACHE:
        return _BUILD_CACHE[key]

    from contextlib import ExitStack
    import concourse.tile as tile
    from concourse import bacc, mybir

    f32 = mybir.dt.float32
    f16 = mybir.dt.float16
    ccmax = max(cc_p)
    gw_p, nboff, msoff, sbs = _schedule(cc_p)
    totf = int(msoff[-1])
    sb_max = max(int(msoff[sb[-1] + 1] - msoff[sb[0]]) for sb in sbs)
    nt_max = max(int(nboff[sb[-1] + 1] - nboff[sb[0]]) for sb in sbs)

    nc = bacc.Bacc("TRN2", target_bir_lowering=False, debug=False,
                   num_devices=NCORES)

    msgs = nc.dram_tensor("msgs", [D, totf], f16, kind="ExternalInput").ap()
    embedT = nc.dram_tensor("embedT", [D, NPC_PAD], f16,
                            kind="ExternalInput").ap()
    w1 = nc.dram_tensor("w1", [D, D], f16, kind="ExternalInput").ap()
    w2 = nc.dram_tensor("w2", [D, D], f16, kind="ExternalInput").ap()
    ident = nc.dram_tensor("ident", [D, D], f16, kind="ExternalInput").ap()
    b1 = nc.dram_tensor("b1", [D, 1], f32, kind="ExternalInput").ap()
    b2 = nc.dram_tensor("b2", [D, 1], f32, kind="ExternalInput").ap()
    outT = nc.dram_tensor("outT", [D, NPC_PAD], f16,
                          kind="ExternalOutput").ap()

    tree_sizes = []
    n = ccmax
    while n > 1:
        h = n // 2
        tree_sizes.append(h)
        n = h + (n & 1)

    with tile.TileContext(nc) as tc, ExitStack() as ctx:
        const = ctx.enter_context(tc.tile_pool(name="const", bufs=1))
        msgp = ctx.enter_context(tc.tile_pool(name="msg", bufs=2))
        ntp = ctx.enter_context(tc.tile_pool(name="ntp", bufs=2))
        trp = ctx.enter_context(tc.tile_pool(name="tree", bufs=2))
        xp = ctx.enter_context(tc.tile_pool(name="xp", bufs=4))
        rp = ctx.enter_context(tc.tile_pool(name="rp", bufs=4))
        op = ctx.enter_context(tc.tile_pool(name="op", bufs=3))
        psout = ctx.enter_context(tc.tile_pool(name="psout", bufs=2, space="PSUM"))

        lrelu = mybir.ActivationFunctionType.Lrelu
        add = mybir.AluOpType.add
        mult = mybir.AluOpType.mult

        # first superblock's data before anything else: compute starts early
        sb_tiles = {}

        def load_sb(s):
            sb = sbs[s]
            mlo, mhi = int(msoff[sb[0]]), int(msoff[sb[-1] + 1])
            nlo, nhi = int(nboff[sb[0]]), int(nboff[sb[-1] + 1])
            msb = msgp.tile([D, sb_max], f16, tag="msg")
            nc.sync.dma_start(msb[:, : mhi - mlo], msgs[:, mlo:mhi])
            ntb = ntp.tile([D, nt_max], f16, tag="nt")
            nc.sync.dma_start(ntb[:, : nhi - nlo], embedT[:, nlo:nhi])
            sb_tiles[s] = (msb, mlo, ntb, nlo)

        load_sb(0)

        w1_sb = const.tile([D, D], f16)
        nc.sync.dma_start(w1_sb[:], w1)
        w2_sb = const.tile([D, D], f16)
        nc.sync.dma_start(w2_sb[:], w2)
        id_sb = const.tile([D, D], f16)
        nc.sync.dma_start(id_sb[:], ident)
        b1_sb = const.tile([D, 1], f32)
        nc.sync.dma_start(b1_sb[:], b1)
        b2_sb = const.tile([D, 1], f32)
        nc.sync.dma_start(b2_sb[:], b2)

        state = {}

        def stage_a(i, msb, mbase, ntb, nbase):
            """tree -> nh; x2 on gpsimd; o1 fold + r1."""
            cc = int(cc_p[i])
            gw = gw_p[i]
            planes = (msb, int(msoff[i]) - mbase, cc)
            carry = []
            lvl = 0
            while planes[2] > 1 or (planes[2] == 1 and carry):
                tile_, co, n = planes
                if n == 1:
                    a = carry.pop()
                    dst = trp.tile([D, GW], f16, tag="carry")
                    nc.vector.tensor_tensor(out=dst[:, :gw],
                                            in0=tile_[:, co : co + gw],
                                            in1=a, op=add)
                    planes = (dst, 0, 1)
                    continue
                h = n // 2
                dst = trp.tile(
                    [D, tree_sizes[min(lvl, len(tree_sizes) - 1)] * GW],
                    f16, tag=f"t{lvl}")
                nc.vector.tensor_tensor(
                    out=dst[:, : h * gw],
                    in0=tile_[:, co : co + h * gw],
                    in1=tile_[:, co + h * gw : co + 2 * h * gw],
                    op=add)
                if n & 1:
                    carry.append(tile_[:, co + 2 * h * gw : co + n * gw])
                planes = (dst, 0, h)
                lvl += 1
            nh_t, nh_co, _ = planes
            nh = nh_t[:, nh_co : nh_co + gw]

            nt = ntb[:, int(nboff[i]) - nbase : int(nboff[i]) - nbase + gw]
            x2 = xp.tile([D, GW], f16, tag="x2")
            nc.gpsimd.tensor_tensor(out=x2[:, :gw], in0=nt, in1=nh, op=mult)

            o1 = psout.tile([D, GW], f32, tag="o1")
            nc.tensor.matmul(out=o1[:, :gw], lhsT=w1_sb[:], rhs=nt,
                             start=True, stop=False)
            nc.tensor.matmul(out=o1[:, :gw], lhsT=w1_sb[:], rhs=nh,
                             start=False, stop=True)
            r1 = rp.tile([D, GW], f16, tag="r1")
            nc.scalar.activation(out=r1[:, :gw], in_=o1[:, :gw],
                                 func=lrelu, bias=b1_sb[:], scale=1.0,
                                 alpha=0.01)
            state[i] = dict(gw=gw, x2=x2, r1=r1)

        def stage_b(i):
            """o2 = W2^T @ x2 (one group late), r2."""
            st = state[i]
            gw = st["gw"]
            o2 = psout.tile([D, GW], f32, tag="o2")
            nc.tensor.matmul(out=o2[:, :gw], lhsT=w2_sb[:],
                             rhs=st["x2"][:, :gw], start=True, stop=True)
            r2 = rp.tile([D, GW], f16, tag="r2")
            nc.scalar.activation(out=r2[:, :gw], in_=o2[:, :gw],
                                 func=lrelu, bias=b2_sb[:], scale=1.0,
                                 alpha=0.01)
            st["r2"] = r2

        def stage_c(i):
            """r1+r2 on PE (identity accumulate), PSUM->SBUF, store."""
            st = state.pop(i)
            gw = st["gw"]
            po = psout.tile([D, GW], f32, tag="po")
            nc.tensor.matmul(out=po[:, :gw], lhsT=id_sb[:],
                             rhs=st["r1"][:, :gw], start=True, stop=False)
            nc.tensor.matmul(out=po[:, :gw], lhsT=id_sb[:],
                             rhs=st["r2"][:, :gw], start=False, stop=True)
            ot = op.tile([D, GW], f16, tag="ot")
            nc.scalar.copy(out=ot[:, :gw], in_=po[:, :gw])
            nc.sync.dma_start(
                outT[:, int(nboff[i]) : int(nboff[i]) + gw], ot[:, :gw])

        with nc.allow_low_precision("fp16 pipeline; f32 PSUM accumulate"):
            done = []
            for s, sb in enumerate(sbs):
                if s > 0:
                    load_sb(s)
                msb, mbase, ntb, nbase = sb_tiles.pop(s)
                for i in sb:
                    if len(done) >= 1:
                        stage_b(done[-1])
                    if len(done) >= 2:
                        stage_c(done[-2])
                    stage_a(i, msb, mbase, ntb, nbase)
                    done.append(i)
            stage_b(done[-1])
            stage_c(done[-2])
            stage_c(done[-1])

    nc.compile()
    _BUILD_CACHE[key] = nc
    return nc


def _core_meta(c, dst):
    """Ascending-degree final ranks for one core + per-position max deg."""
    mask = (dst >= c * NPC) & (dst < (c + 1) * NPC)
    ld = (dst[mask] - c * NPC).astype(np.int64)
    deg = np.bincount(ld, minlength=NPC)
    asc = np.argsort(deg, kind="stable")         # real nodes, deg ascending
    # natural ranks: pads (deg 0) first, then ascending-degree real nodes
    node_nat = np.concatenate([np.full(NPAD, -1, np.int64), asc])
    deg_nat = np.where(node_nat >= 0, deg[np.maximum(node_nat, 0)], 0)
    widths = _nat_widths()
    wb = np.concatenate(([0], np.cumsum(widths))).astype(np.int64)
    perm = _proc_perm()
    node_fin = np.concatenate([node_nat[wb[j] : wb[j + 1]] for j in perm])
    deg_fin = np.concatenate([deg_nat[wb[j] : wb[j + 1]] for j in perm])
    gw_p = np.asarray([widths[j] for j in perm], np.int64)
    pb = np.concatenate(([0], np.cumsum(gw_p))).astype(np.int64)
    cc_p = np.asarray([deg_fin[pb[i] : pb[i + 1]].max() for i in range(NG)])
    return node_fin, deg_fin, cc_p


def _prep_core(c, meta, src, dst, att_flat, entity_embed, cc_p):
    """Host-side packing for one core. Returns the per-core input map."""
    node_fin, deg_fin, _ = meta
    gw_p, nboff, msoff, _ = _schedule(cc_p)
    gw_p = np.asarray(gw_p, np.int64)

    mask = (dst >= c * NPC) & (dst < (c + 1) * NPC)
    ld = (dst[mask] - c * NPC).astype(np.int64)
    e_src = src[mask]
    e_att = att_flat[mask]

    fr_of_node = np.empty(NPC, np.int64)
    real = node_fin >= 0
    fr_of_node[node_fin[real]] = np.nonzero(real)[0]
    er = fr_of_node[ld]                          # edge -> final dst rank

    order = np.argsort(er, kind="stable")
    er_s = er[order]
    starts_all = np.zeros(NPC_PAD + 1, np.int64)
    cnt = np.bincount(er_s, minlength=NPC_PAD)
    starts_all[1:] = np.cumsum(cnt)
    occ = np.arange(len(er_s)) - starts_all[er_s]

    pos = np.searchsorted(nboff, er_s, side="right") - 1
    i_in = er_s - nboff[pos]
    cols = msoff[pos] + occ * gw_p[pos] + i_in   # plane-major, proc order

    prod = (entity_embed[e_src[order]] * e_att[order, None]).astype(np.float16)
    arr = np.zeros((int(msoff[-1]), D), np.float16)
    arr[cols] = prod
    msgs = np.ascontiguousarray(arr.T)           # [D, TOTF]

    ep = np.zeros((NPC_PAD, D), np.float16)
    ep[real] = entity_embed[c * NPC + node_fin[real]]
    embedT = np.ascontiguousarray(ep.T)          # [D, NPC_PAD]

    return dict(msgs=msgs, embedT=embedT)


def kernel(entity_embed, att, W1, b1, W2, b2, src, dst):
    from concourse.bass_utils import run_bass_kernel_spmd

    entity_embed = np.ascontiguousarray(np.asarray(entity_embed, dtype=np.float32))
    att_flat = np.asarray(att, dtype=np.float32).reshape(-1)
    W1h = np.asarray(W1, dtype=np.float16)
    W2h = np.asarray(W2, dtype=np.float16)
    b1c = np.asarray(b1, dtype=np.float32).reshape(D, 1)
    b2c = np.asarray(b2, dtype=np.float32).reshape(D, 1)
    src = np.asarray(src).astype(np.int64)
    dst = np.asarray(dst).astype(np.int64)

    metas = [_core_meta(c, dst) for c in range(NCORES)]
    cc_p = np.maximum(np.stack([m[2] for m in metas]).max(axis=0), 1)
    cc_p = cc_p.astype(np.int64)

    shared = dict(w1=W1h, w2=W2h, b1=b1c, b2=b2c,
                  ident=np.eye(D, dtype=np.float16))
    in_maps = []
    for c in range(NCORES):
        m = _prep_core(c, metas[c], src, dst, att_flat, entity_embed, cc_p)
        m.update(shared)
        in_maps.append(m)

    nc = _build(cc_p)
    res = run_bass_kernel_spmd(nc, in_maps, core_ids=list(range(NCORES)))

    out = np.empty((N_NODES, D), np.float32)
    for c in range(NCORES):
        o = res.results[c]["outT"]               # [128d, NPC_PAD] fp16
        o = o.T.astype(np.float32)               # [NPC_PAD, 128]
        node_fin = metas[c][0]
        real = node_fin >= 0
        blk = out[c * NPC : (c + 1) * NPC]
        blk[node_fin[real]] = o[real]
    return out


# revision 17
# speedup vs baseline: 1.0938x; 1.0938x over previous
"""TRN2 Bass kernel for nn_Aggregator (GNN message passing + bi-interaction).

Computes, for graph with N=100000 nodes, E=800000 edges, D=128:
    msgs = entity_embed[src] * att                  (per-edge message)
    N_h  = segment_sum(msgs, dst)                   (scatter-add to nodes)
    out  = LRelu((node+N_h)@W1+b1) + LRelu((node*N_h)@W2+b2)

Strategy (8 NeuronCores, SPMD, no collectives):
  * Edges are bucketed by dst//12500 -> owning core; each core computes the
    full output rows for its 12500-node partition.
  * The host materializes per-edge messages (embed[src]*att, fp16) into a
    degree-sorted slotted layout -- the sharding hint's "messages" input.
    Nodes are sorted by in-degree ASCENDING (pads first) and renumbered so
    group processing order == memory order: group i covers final ranks
    [nboff[i], nboff[i]+gw), needs CC[i] = max in-group degree occurrence
    planes, and its messages live plane-major at
    col = msoff[i] + c*gw + rank_in_group.  The two biggest-CC groups are
    moved a few slots before the end so the pipeline tail is short.
  * Device segment-sum = binary halving tree of contiguous fp16 DVE
    tensor_tensor adds (~1.0ns/col; tensor_reduce measured 1.05 with no
    grouping freedom).  No gather (the old SWDGE dma_gather serialized
    ~386us of descriptor generation on GpSimd), no one-hot matmul.
  * msgs and the node table stream in ramped superblocks (0.5MB..4MB, ~8
    dma_starts each instead of 25/1) to amortize the ~2us fixed per-DMA
    completion latency and ride the 436 GB/s DMA setup-knee; msgs for the
    first superblock are issued before everything else so compute starts
    ~6us in.
  * x1 = node+N_h is never materialized: PE computes
    o1 = W1^T@nodeT + W1^T@N_hT by PSUM accumulation (fp16 matmuls, f32
    accumulate -- also kills one fp16 rounding).  x2 = nodeT*N_hT on
    GpSimd (its port never contends with DVE tensor_tensor), o2 deferred
    one group so the PE in-order queue never waits on GpSimd;
    bias+LeakyReLU on Scalar (f32 bias APs).  The final r1+r2 runs on PE
    as two accumulating identity matmuls (deferred two groups), Scalar
    copies PSUM->SBUF, fp16 outT stored per group.
  * The host inverse-permutes and upcasts at the end.  The CC schedule is
    shared across cores (SPMD single program), per-group max over cores
    (total slots ~ E/8 + ~6%).
"""
import sys

sys.path.insert(0, "/opt/trn_rl_repo")

import numpy as np

N_NODES = 100000
N_EDGES = 800000
D = 128
NCORES = 8
NPC = N_NODES // NCORES          # 12500 nodes per core
W = 128
NWIN = (NPC + W - 1) // W        # 98 windows per core
NPC_PAD = NWIN * W               # padded node count per core (12544)
NPAD = NPC_PAD - NPC             # 44 pad nodes (rank 0.., zero embed)
GRP = 4
GW = GRP * W                     # 512 node columns per group
NG = (NWIN + GRP - 1) // GRP     # 25 groups (one is 256 wide)
SB_RAMP = (1024, 2048, 4096)     # first superblock slot budgets
SB_CAP = 8192                    # steady-state superblock slots (16KB/part)
SB_GWCAP = 2048                  # max node columns per superblock

_BUILD_CACHE = {}


def _nat_widths():
    """Natural group widths: 24x512 + 1x256 (the top-degree group)."""
    return [GW] * (NG - 1) + [NPC_PAD - (NG - 1) * GW]


def _proc_perm():
    """Processing order of natural groups: ascending CC (= natural order)
    except groups 2,3 (small CC) are saved for last — the pipeline drain
    then trails a ~1-plane group instead of the max-CC one."""
    if NG < 6:
        return list(range(NG))
    return [0, 1] + list(range(4, NG)) + [2, 3]


def _schedule(cc_p):
    """Superblocks + offsets over processed positions.

    cc_p[i] = occurrence planes of processed group i.
    Returns (gw_p, nboff, msoff, superblocks as lists of positions).
    """
    widths = _nat_widths()
    perm = _proc_perm()
    gw_p = [widths[j] for j in perm]
    nboff = np.concatenate(([0], np.cumsum(gw_p))).astype(np.int64)
    slots = [int(cc_p[i]) * gw_p[i] for i in range(NG)]
    msoff = np.concatenate(([0], np.cumsum(slots))).astype(np.int64)
    sbs, cur, cur_slots, cur_gw, ramp = [], [], 0, 0, 0
    for i in range(NG):
        cap = SB_RAMP[ramp] if ramp < len(SB_RAMP) else SB_CAP
        if cur and (cur_slots + slots[i] > cap or cur_gw + gw_p[i] > SB_GWCAP):
            sbs.append(cur)
            cur, cur_slots, cur_gw = [], 0, 0
            ramp += 1
        cur.append(i)
        cur_slots += slots[i]
        cur_gw += gw_p[i]
    if cur:
        sbs.append(cur)
    return gw_p, nboff, msoff, sbs


def _build(cc_p):
    """Build + bacc-compile the SPMD Bass program for a CC schedule."""
    key = tuple(cc_p)
    if key in _BUILD_CACHE:
        return _BUILD_CACHE[key]

    from contextlib import ExitStack
    import concourse.tile as tile
    from concourse import bacc, mybir

    f32 = mybir.dt.float32
    f16 = mybir.dt.float16
    ccmax = max(cc_p)
    gw_p, nboff, msoff, sbs = _schedule(cc_p)
    totf = int(msoff[-1])
    sb_max = max(int(msoff[sb[-1] + 1] - msoff[sb[0]]) for sb in sbs)
    nt_max = max(int(nboff[sb[-1] + 1] - nboff[sb[0]]) for sb in sbs)

    nc = bacc.Bacc("TRN2", target_bir_lowering=False, debug=False,
                   num_devices=NCORES)

    msgs = nc.dram_tensor("msgs", [D, totf], f16, kind="ExternalInput").ap()
    embedT = nc.dram_tensor("embedT", [D, NPC_PAD], f16,
                            kind="ExternalInput").ap()
    w1 = nc.dram_tensor("w1", [D, D], f16, kind="ExternalInput").ap()
    w2 = nc.dram_tensor("w2", [D, D], f16, kind="ExternalInput").ap()
    b1 = nc.dram_tensor("b1", [D, 1], f32, kind="ExternalInput").ap()
    b2 = nc.dram_tensor("b2", [D, 1], f32, kind="ExternalInput").ap()
    outT = nc.dram_tensor("outT", [D, NPC_PAD], f16,
                          kind="ExternalOutput").ap()

    tree_sizes = []
    n = ccmax
    while n > 1:
        h = n // 2
        tree_sizes.append(h)
        n = h + (n & 1)

    with tile.TileContext(nc) as tc, ExitStack() as ctx:
        const = ctx.enter_context(tc.tile_pool(name="const", bufs=1))
        msgp = ctx.enter_context(tc.tile_pool(name="msg", bufs=4))
        ntp = ctx.enter_context(tc.tile_pool(name="ntp", bufs=4))
        trp = ctx.enter_context(tc.tile_pool(name="tree", bufs=2))
        xp = ctx.enter_context(tc.tile_pool(name="xp", bufs=4))
        rp = ctx.enter_context(tc.tile_pool(name="rp", bufs=4))
        op = ctx.enter_context(tc.tile_pool(name="op", bufs=3))
        psout = ctx.enter_context(tc.tile_pool(name="psout", bufs=2, space="PSUM"))

        lrelu = mybir.ActivationFunctionType.Lrelu
        add = mybir.AluOpType.add
        mult = mybir.AluOpType.mult

        # first superblock's data before anything else: compute starts early
        sb_tiles = {}

        def load_sb(s):
            sb = sbs[s]
            mlo, mhi = int(msoff[sb[0]]), int(msoff[sb[-1] + 1])
            nlo, nhi = int(nboff[sb[0]]), int(nboff[sb[-1] + 1])
            msb = msgp.tile([D, sb_max], f16, tag="msg")
            nc.sync.dma_start(msb[:, : mhi - mlo], msgs[:, mlo:mhi])
            ntb = ntp.tile([D, nt_max], f16, tag="nt")
            nc.sync.dma_start(ntb[:, : nhi - nlo], embedT[:, nlo:nhi])
            sb_tiles[s] = (msb, mlo, ntb, nlo)

        load_sb(0)

        w1_sb = const.tile([D, D], f16)
        nc.sync.dma_start(w1_sb[:], w1)
        w2_sb = const.tile([D, D], f16)
        nc.sync.dma_start(w2_sb[:], w2)
        b1_sb = const.tile([D, 1], f32)
        nc.sync.dma_start(b1_sb[:], b1)
        b2_sb = const.tile([D, 1], f32)
        nc.sync.dma_start(b2_sb[:], b2)

        state = {}

        def stage_a(i, msb, mbase, ntb, nbase):
            """tree -> nh; x2 on DVE (same queue as tree -- free edge);
            o1 fold + o2 on PE; r1/r2 on Scalar."""
            cc = int(cc_p[i])
            gw = gw_p[i]
            planes = (msb, int(msoff[i]) - mbase, cc)
            carry = []
            lvl = 0
            while planes[2] > 1 or (planes[2] == 1 and carry):
                tile_, co, n = planes
                if n == 1:
                    a = carry.pop()
                    dst = trp.tile([D, GW], f16, tag="carry")
                    nc.vector.tensor_tensor(out=dst[:, :gw],
                                            in0=tile_[:, co : co + gw],
                                            in1=a, op=add)
                    planes = (dst, 0, 1)
                    continue
                h = n // 2
                dst = trp.tile(
                    [D, tree_sizes[min(lvl, len(tree_sizes) - 1)] * GW],
                    f16, tag=f"t{lvl}")
                nc.vector.tensor_tensor(
                    out=dst[:, : h * gw],
                    in0=tile_[:, co : co + h * gw],
                    in1=tile_[:, co + h * gw : co + 2 * h * gw],
                    op=add)
                if n & 1:
                    carry.append(tile_[:, co + 2 * h * gw : co + n * gw])
                planes = (dst, 0, h)
                lvl += 1
            nh_t, nh_co, _ = planes
            nh = nh_t[:, nh_co : nh_co + gw]

            nt = ntb[:, int(nboff[i]) - nbase : int(nboff[i]) - nbase + gw]
            x2 = xp.tile([D, GW], f16, tag="x2")
            nc.vector.tensor_tensor(out=x2[:, :gw], in0=nt, in1=nh, op=mult)

            o1 = psout.tile([D, GW], f32, tag="o1")
            nc.tensor.matmul(out=o1[:, :gw], lhsT=w1_sb[:], rhs=nt,
                             start=True, stop=False)
            nc.tensor.matmul(out=o1[:, :gw], lhsT=w1_sb[:], rhs=nh,
                             start=False, stop=True)
            o2 = psout.tile([D, GW], f32, tag="o2")
            nc.tensor.matmul(out=o2[:, :gw], lhsT=w2_sb[:],
                             rhs=x2[:, :gw], start=True, stop=True)
            r1 = rp.tile([D, GW], f16, tag="r1")
            nc.scalar.activation(out=r1[:, :gw], in_=o1[:, :gw],
                                 func=lrelu, bias=b1_sb[:], scale=1.0,
                                 alpha=0.01)
            r2 = rp.tile([D, GW], f16, tag="r2")
            nc.scalar.activation(out=r2[:, :gw], in_=o2[:, :gw],
                                 func=lrelu, bias=b2_sb[:], scale=1.0,
                                 alpha=0.01)
            state[i] = dict(gw=gw, r1=r1, r2=r2)

        def stage_fin(i):
            """r1+r2 on GpSimd (one group late), store via Scalar HWDGE."""
            st = state.pop(i)
            gw = st["gw"]
            ot = op.tile([D, GW], f16, tag="ot")
            nc.gpsimd.tensor_tensor(out=ot[:, :gw], in0=st["r1"][:, :gw],
                                    in1=st["r2"][:, :gw], op=add)
            nc.scalar.dma_start(
                outT[:, int(nboff[i]) : int(nboff[i]) + gw], ot[:, :gw])

        with nc.allow_low_precision("fp16 pipeline; f32 PSUM accumulate"):
            done = []
            for s, sb in enumerate(sbs):
                if s > 0:
                    load_sb(s)
                msb, mbase, ntb, nbase = sb_tiles.pop(s)
                for i in sb:
                    if len(done) >= 1:
                        stage_fin(done[-1])
                    stage_a(i, msb, mbase, ntb, nbase)
                    done.append(i)
            stage_fin(done[-1])

    nc.compile()
    _BUILD_CACHE[key] = nc
    return nc


def _core_meta(c, dst):
    """Ascending-degree final ranks for one core + per-position max deg."""
    mask = (dst >= c * NPC) & (dst < (c + 1) * NPC)
    ld = (dst[mask] - c * NPC).astype(np.int64)
    deg = np.bincount(ld, minlength=NPC)
    asc = np.argsort(deg, kind="stable")         # real nodes, deg ascending
    # natural ranks: pads (deg 0) first, then ascending-degree real nodes
    node_nat = np.concatenate([np.full(NPAD, -1, np.int64), asc])
    deg_nat = np.where(node_nat >= 0, deg[np.maximum(node_nat, 0)], 0)
    widths = _nat_widths()
    wb = np.concatenate(([0], np.cumsum(widths))).astype(np.int64)
    perm = _proc_perm()
    node_fin = np.concatenate([node_nat[wb[j] : wb[j + 1]] for j in perm])
    deg_fin = np.concatenate([deg_nat[wb[j] : wb[j + 1]] for j in perm])
    gw_p = np.asarray([widths[j] for j in perm], np.int64)
    pb = np.concatenate(([0], np.cumsum(gw_p))).astype(np.int64)
    cc_p = np.asarray([deg_fin[pb[i] : pb[i + 1]].max() for i in range(NG)])
    return node_fin, deg_fin, cc_p


def _prep_core(c, meta, src, dst, att_flat, entity_embed, cc_p):
    """Host-side packing for one core. Returns the per-core input map."""
    node_fin, deg_fin, _ = meta
    gw_p, nboff, msoff, _ = _schedule(cc_p)
    gw_p = np.asarray(gw_p, np.int64)

    mask = (dst >= c * NPC) & (dst < (c + 1) * NPC)
    ld = (dst[mask] - c * NPC).astype(np.int64)
    e_src = src[mask]
    e_att = att_flat[mask]

    fr_of_node = np.empty(NPC, np.int64)
    real = node_fin >= 0
    fr_of_node[node_fin[real]] = np.nonzero(real)[0]
    er = fr_of_node[ld]                          # edge -> final dst rank

    order = np.argsort(er, kind="stable")
    er_s = er[order]
    starts_all = np.zeros(NPC_PAD + 1, np.int64)
    cnt = np.bincount(er_s, minlength=NPC_PAD)
    starts_all[1:] = np.cumsum(cnt)
    occ = np.arange(len(er_s)) - starts_all[er_s]

    pos = np.searchsorted(nboff, er_s, side="right") - 1
    i_in = er_s - nboff[pos]
    cols = msoff[pos] + occ * gw_p[pos] + i_in   # plane-major, proc order

    prod = (entity_embed[e_src[order]] * e_att[order, None]).astype(np.float16)
    arr = np.zeros((int(msoff[-1]), D), np.float16)
    arr[cols] = prod
    msgs = np.ascontiguousarray(arr.T)           # [D, TOTF]

    ep = np.zeros((NPC_PAD, D), np.float16)
    ep[real] = entity_embed[c * NPC + node_fin[real]]
    embedT = np.ascontiguousarray(ep.T)          # [D, NPC_PAD]

    return dict(msgs=msgs, embedT=embedT)


def kernel(entity_embed, att, W1, b1, W2, b2, src, dst):
    from concourse.bass_utils import run_bass_kernel_spmd

    entity_embed = np.ascontiguousarray(np.asarray(entity_embed, dtype=np.float32))
    att_flat = np.asarray(att, dtype=np.float32).reshape(-1)
    W1h = np.asarray(W1, dtype=np.float16)
    W2h = np.asarray(W2, dtype=np.float16)
    b1c = np.asarray(b1, dtype=np.float32).reshape(D, 1)
    b2c = np.asarray(b2, dtype=np.float32).reshape(D, 1)
    src = np.asarray(src).astype(np.int64)
    dst = np.asarray(dst).astype(np.int64)

    metas = [_core_meta(c, dst) for c in range(NCORES)]
    cc_p = np.maximum(np.stack([m[2] for m in metas]).max(axis=0), 1)
    cc_p = cc_p.astype(np.int64)

    shared = dict(w1=W1h, w2=W2h, b1=b1c, b2=b2c)
    in_maps = []
    for c in range(NCORES):
        m = _prep_core(c, metas[c], src, dst, att_flat, entity_embed, cc_p)
        m.update(shared)
        in_maps.append(m)

    nc = _build(cc_p)
    res = run_bass_kernel_spmd(nc, in_maps, core_ids=list(range(NCORES)))

    out = np.empty((N_NODES, D), np.float32)
    for c in range(NCORES):
        o = res.results[c]["outT"]               # [128d, NPC_PAD] fp16
        o = o.T.astype(np.float32)               # [NPC_PAD, 128]
        node_fin = metas[c][0]
        real = node_fin >= 0
        blk = out[c * NPC : (c + 1) * NPC]
        blk[node_fin[real]] = o[real]
    return out


# revision 25
# speedup vs baseline: 1.1434x; 1.0454x over previous
"""TRN2 Bass kernel for nn_Aggregator (GNN message passing + bi-interaction).

Computes, for graph with N=100000 nodes, E=800000 edges, D=128:
    msgs = entity_embed[src] * att                  (per-edge message)
    N_h  = segment_sum(msgs, dst)                   (scatter-add to nodes)
    out  = LRelu((node+N_h)@W1+b1) + LRelu((node*N_h)@W2+b2)

Strategy (8 NeuronCores, SPMD, no collectives):
  * Edges are bucketed by dst//12500 -> owning core; each core computes the
    full output rows for its 12500-node partition.
  * The host materializes per-edge messages (embed[src]*att, fp16) into a
    degree-sorted slotted layout -- the sharding hint's "messages" input.
    Nodes are sorted by in-degree ASCENDING (pads first) and renumbered so
    group processing order == memory order: group i covers final ranks
    [nboff[i], nboff[i]+gw), needs CC[i] = max in-group degree occurrence
    planes, and its messages live plane-major at
    col = msoff[i] + c*gw + rank_in_group.  The two biggest-CC groups are
    moved a few slots before the end so the pipeline tail is short.
  * Device segment-sum = binary halving tree of contiguous fp16 DVE
    tensor_tensor adds (~1.0ns/col; tensor_reduce measured 1.05 with no
    grouping freedom).  No gather (the old SWDGE dma_gather serialized
    ~386us of descriptor generation on GpSimd), no one-hot matmul.
  * msgs and the node table stream in ramped superblocks (0.5MB..4MB, ~8
    dma_starts each instead of 25/1) to amortize the ~2us fixed per-DMA
    completion latency and ride the 436 GB/s DMA setup-knee; msgs for the
    first superblock are issued before everything else so compute starts
    ~6us in.
  * x1 = node+N_h is never materialized: PE computes
    o1 = W1^T@nodeT + W1^T@N_hT by PSUM accumulation (fp16 matmuls, f32
    accumulate -- also kills one fp16 rounding).  x2 = nodeT*N_hT on
    GpSimd (its port never contends with DVE tensor_tensor), o2 deferred
    one group so the PE in-order queue never waits on GpSimd;
    bias+LeakyReLU on Scalar (f32 bias APs).  The final r1+r2 runs on PE
    as two accumulating identity matmuls (deferred two groups), Scalar
    copies PSUM->SBUF, fp16 outT stored per group.
  * The host inverse-permutes and upcasts at the end.  The CC schedule is
    shared across cores (SPMD single program), per-group max over cores
    (total slots ~ E/8 + ~6%).
"""
import sys

sys.path.insert(0, "/opt/trn_rl_repo")

import numpy as np

N_NODES = 100000
N_EDGES = 800000
D = 128
NCORES = 8
NPC = N_NODES // NCORES          # 12500 nodes per core
W = 128
NWIN = (NPC + W - 1) // W        # 98 windows per core
NPC_PAD = NWIN * W               # padded node count per core (12544)
NPAD = NPC_PAD - NPC             # 44 pad nodes (rank 0.., zero embed)
GRP = 4
GW = GRP * W                     # 512 node columns per group
NG = (NWIN + GRP - 1) // GRP     # 25 groups (one is 256 wide)
SB_RAMP = (1024, 2048, 4096)     # first superblock slot budgets
SB_CAP = 8192                    # steady-state superblock slots (16KB/part)
SB_GWCAP = 2048                  # max node columns per superblock

_BUILD_CACHE = {}


def _nat_widths():
    """Natural group widths: 24x512 + 1x256 (the top-degree group)."""
    return [GW] * (NG - 1) + [NPC_PAD - (NG - 1) * GW]


def _proc_perm():
    """Processing order of natural groups: ascending CC (= natural order)
    except groups 2,3 (small CC) are saved for last — the pipeline drain
    then trails a ~1-plane group instead of the max-CC one."""
    if NG < 6:
        return list(range(NG))
    return [0, 1] + list(range(4, NG)) + [2, 3]


def _schedule(cc_p):
    """Superblocks + offsets over processed positions.

    cc_p[i] = occurrence planes of processed group i.  Each group's planes
    are split into an A family (first ceil(cc/2)) and a B family (rest) so
    every tree-level add streams its two operands from two different SBUF
    tiles (measured ~0.53ns/col vs ~0.85 same-tile).  The msgs DRAM image
    is, per superblock: [A planes of its groups | B planes of its groups].

    Returns (gw_p, nboff, superblocks, na/nb, aoff/boff local col offsets,
    sb A/B base offsets and sizes).
    """
    widths = _nat_widths()
    perm = _proc_perm()
    gw_p = [widths[j] for j in perm]
    nboff = np.concatenate(([0], np.cumsum(gw_p))).astype(np.int64)
    na = [(int(cc_p[i]) + 1) // 2 for i in range(NG)]
    nb = [int(cc_p[i]) - na[i] for i in range(NG)]
    slots = [int(cc_p[i]) * gw_p[i] for i in range(NG)]
    sbs, cur, cur_slots, cur_gw, ramp = [], [], 0, 0, 0
    for i in range(NG):
        cap = SB_RAMP[ramp] if ramp < len(SB_RAMP) else SB_CAP
        if cur and (cur_slots + slots[i] > cap or cur_gw + gw_p[i] > SB_GWCAP):
            sbs.append(cur)
            cur, cur_slots, cur_gw = [], 0, 0
            ramp += 1
        cur.append(i)
        cur_slots += slots[i]
        cur_gw += gw_p[i]
    if cur:
        sbs.append(cur)
    aoff = [0] * NG              # col offset of group's A block within sb A
    boff = [0] * NG
    sb_base = []                 # (a_base, a_size, b_base, b_size) in DRAM
    pos = 0
    for sb in sbs:
        asz = sum(na[i] * gw_p[i] for i in sb)
        bsz = sum(nb[i] * gw_p[i] for i in sb)
        a = 0
        b = 0
        for i in sb:
            aoff[i] = a
            boff[i] = b
            a += na[i] * gw_p[i]
            b += nb[i] * gw_p[i]
        sb_base.append((pos, asz, pos + asz, bsz))
        pos += asz + bsz
    return gw_p, nboff, sbs, na, nb, aoff, boff, sb_base, pos


def _build(cc_p):
    """Build + bacc-compile the SPMD Bass program for a CC schedule."""
    key = tuple(cc_p)
    if key in _BUILD_CACHE:
        return _BUILD_CACHE[key]

    from contextlib import ExitStack
    import concourse.tile as tile
    from concourse import bacc, mybir

    f32 = mybir.dt.float32
    f16 = mybir.dt.float16
    ccmax = max(cc_p)
    gw_p, nboff, sbs, na, nb, aoff, boff, sb_base, totf = _schedule(cc_p)
    sba_max = max(a for (_, a, _, _) in sb_base)
    sbb_max = max(b for (_, _, _, b) in sb_base)
    nt_max = max(int(nboff[sb[-1] + 1] - nboff[sb[0]]) for sb in sbs)

    nc = bacc.Bacc("TRN2", target_bir_lowering=False, debug=False,
                   num_devices=NCORES)

    msgs = nc.dram_tensor("msgs", [D, totf], f16, kind="ExternalInput").ap()
    embedT = nc.dram_tensor("embedT", [D, NPC_PAD], f16,
                            kind="ExternalInput").ap()
    w1 = nc.dram_tensor("w1", [D, D], f16, kind="ExternalInput").ap()
    w2 = nc.dram_tensor("w2", [D, D], f16, kind="ExternalInput").ap()
    b1 = nc.dram_tensor("b1", [D, 1], f32, kind="ExternalInput").ap()
    b2 = nc.dram_tensor("b2", [D, 1], f32, kind="ExternalInput").ap()
    outT = nc.dram_tensor("outT", [D, NPC_PAD], f16,
                          kind="ExternalOutput").ap()

    # per-level A/B plane counts for scratch sizing (worst case = ccmax)
    lvl_sizes = []               # (newA planes, newB planes) per level
    a_n, b_n = (ccmax + 1) // 2, ccmax // 2
    while a_n + b_n > 1:
        m = min(a_n, b_n)
        if m == 0:
            break
        q = (m + 1) // 2
        lvl_sizes.append((q, m - q))
        a_n, b_n = q, m - q

    with tile.TileContext(nc) as tc, ExitStack() as ctx:
        const = ctx.enter_context(tc.tile_pool(name="const", bufs=1))
        msgpa = ctx.enter_context(tc.tile_pool(name="msga", bufs=4))
        msgpb = ctx.enter_context(tc.tile_pool(name="msgb", bufs=4))
        ntp = ctx.enter_context(tc.tile_pool(name="ntp", bufs=4))
        trpa = ctx.enter_context(tc.tile_pool(name="treea", bufs=2))
        trpb = ctx.enter_context(tc.tile_pool(name="treeb", bufs=2))
        xp = ctx.enter_context(tc.tile_pool(name="xp", bufs=4))
        rp = ctx.enter_context(tc.tile_pool(name="rp", bufs=4))
        op = ctx.enter_context(tc.tile_pool(name="op", bufs=3))
        psout = ctx.enter_context(tc.tile_pool(name="psout", bufs=2, space="PSUM"))

        lrelu = mybir.ActivationFunctionType.Lrelu
        add = mybir.AluOpType.add
        mult = mybir.AluOpType.mult

        # first superblock's data before anything else: compute starts early
        sb_tiles = {}

        def load_sb(s):
            sb = sbs[s]
            abase, asz, bbase, bsz = sb_base[s]
            nlo, nhi = int(nboff[sb[0]]), int(nboff[sb[-1] + 1])
            msa = msgpa.tile([D, sba_max], f16, tag="msga")
            nc.sync.dma_start(msa[:, :asz], msgs[:, abase : abase + asz])
            msb_t = None
            if bsz:
                msb_t = msgpb.tile([D, sbb_max], f16, tag="msgb")
                nc.sync.dma_start(msb_t[:, :bsz],
                                  msgs[:, bbase : bbase + bsz])
            ntb = ntp.tile([D, nt_max], f16, tag="nt")
            nc.sync.dma_start(ntb[:, : nhi - nlo], embedT[:, nlo:nhi])
            sb_tiles[s] = (msa, msb_t, ntb, nlo)

        load_sb(0)

        w1_sb = const.tile([D, D], f16)
        nc.sync.dma_start(w1_sb[:], w1)
        w2_sb = const.tile([D, D], f16)
        nc.sync.dma_start(w2_sb[:], w2)
        b1_sb = const.tile([D, 1], f32)
        nc.sync.dma_start(b1_sb[:], b1)
        b2_sb = const.tile([D, 1], f32)
        nc.sync.dma_start(b2_sb[:], b2)

        state = {}

        def stage_a(i, msa, msb_t, ntb, nbase):
            """two-family tree -> nh (every add reads two different tiles);
            x2 on DVE (same queue as tree -- free edge); o1 fold + o2 on
            PE; r1/r2 on Scalar."""
            gw = gw_p[i]
            A = (msa, aoff[i], na[i])            # (tile, col_off, planes)
            B = (msb_t, boff[i], nb[i])
            carries = []                         # odd single planes
            lvl = 0
            while A[2] + B[2] > 1 or carries:
                at, ao, an = A
                bt, bo, bn = B
                if bn == 0:
                    c_t, c_o = carries.pop()
                    dst = (trpa if lvl % 2 else trpb).tile(
                        [D, GW], f16, tag=f"cm{lvl % 2}")
                    nc.vector.tensor_tensor(out=dst[:, :gw],
                                            in0=at[:, ao : ao + gw],
                                            in1=c_t[:, c_o : c_o + gw],
                                            op=add)
                    A = (dst, 0, 1)
                    lvl += 1
                    continue
                m = bn
                if an > m:
                    carries.append((at, ao + m * gw))
                q = (m + 1) // 2
                la, lb = lvl_sizes[min(lvl, len(lvl_sizes) - 1)]
                dsta = trpa.tile([D, max(la, 1) * GW], f16, tag=f"tA{lvl}")
                nc.vector.tensor_tensor(
                    out=dsta[:, : q * gw],
                    in0=at[:, ao : ao + q * gw],
                    in1=bt[:, bo : bo + q * gw], op=add)
                if m - q > 0:
                    dstb = trpb.tile([D, max(lb, 1) * GW], f16,
                                     tag=f"tB{lvl}")
                    nc.vector.tensor_tensor(
                        out=dstb[:, : (m - q) * gw],
                        in0=at[:, ao + q * gw : ao + m * gw],
                        in1=bt[:, bo + q * gw : bo + m * gw], op=add)
                    B = (dstb, 0, m - q)
                else:
                    B = (None, 0, 0)
                A = (dsta, 0, q)
                lvl += 1
            nh_t, nh_co, _ = A
            nh = nh_t[:, nh_co : nh_co + gw]

            nt = ntb[:, int(nboff[i]) - nbase : int(nboff[i]) - nbase + gw]
            x2 = xp.tile([D, GW], f16, tag="x2")
            nc.vector.tensor_tensor(out=x2[:, :gw], in0=nt, in1=nh, op=mult)

            o1 = psout.tile([D, GW], f32, tag="o1")
            nc.tensor.matmul(out=o1[:, :gw], lhsT=w1_sb[:], rhs=nt,
                             start=True, stop=False)
            nc.tensor.matmul(out=o1[:, :gw], lhsT=w1_sb[:], rhs=nh,
                             start=False, stop=True)
            o2 = psout.tile([D, GW], f32, tag="o2")
            nc.tensor.matmul(out=o2[:, :gw], lhsT=w2_sb[:],
                             rhs=x2[:, :gw], start=True, stop=True)
            r1 = rp.tile([D, GW], f16, tag="r1")
            nc.scalar.activation(out=r1[:, :gw], in_=o1[:, :gw],
                                 func=lrelu, bias=b1_sb[:], scale=1.0,
                                 alpha=0.01)
            r2 = rp.tile([D, GW], f16, tag="r2")
            nc.scalar.activation(out=r2[:, :gw], in_=o2[:, :gw],
                                 func=lrelu, bias=b2_sb[:], scale=1.0,
                                 alpha=0.01)
            state[i] = dict(gw=gw, r1=r1, r2=r2)

        def stage_fin(i):
            """r1+r2 on GpSimd (one group late), store via Scalar HWDGE."""
            st = state.pop(i)
            gw = st["gw"]
            ot = op.tile([D, GW], f16, tag="ot")
            nc.gpsimd.tensor_tensor(out=ot[:, :gw], in0=st["r1"][:, :gw],
                                    in1=st["r2"][:, :gw], op=add)
            nc.scalar.dma_start(
                outT[:, int(nboff[i]) : int(nboff[i]) + gw], ot[:, :gw])

        with nc.allow_low_precision("fp16 pipeline; f32 PSUM accumulate"):
            done = []
            for s, sb in enumerate(sbs):
                if s > 0:
                    load_sb(s)
                msa, msb_t, ntb, nbase = sb_tiles.pop(s)
                for i in sb:
                    if len(done) >= 1:
                        stage_fin(done[-1])
                    stage_a(i, msa, msb_t, ntb, nbase)
                    done.append(i)
            stage_fin(done[-1])

    nc.compile()
    _BUILD_CACHE[key] = nc
    return nc


def _core_meta(c, dst):
    """Ascending-degree final ranks for one core + per-position max deg."""
    mask = (dst >= c * NPC) & (dst < (c + 1) * NPC)
    ld = (dst[mask] - c * NPC).astype(np.int64)
    deg = np.bincount(ld, minlength=NPC)
    asc = np.argsort(deg, kind="stable")         # real nodes, deg ascending
    # natural ranks: pads (deg 0) first, then ascending-degree real nodes
    node_nat = np.concatenate([np.full(NPAD, -1, np.int64), asc])
    deg_nat = np.where(node_nat >= 0, deg[np.maximum(node_nat, 0)], 0)
    widths = _nat_widths()
    wb = np.concatenate(([0], np.cumsum(widths))).astype(np.int64)
    perm = _proc_perm()
    node_fin = np.concatenate([node_nat[wb[j] : wb[j + 1]] for j in perm])
    deg_fin = np.concatenate([deg_nat[wb[j] : wb[j + 1]] for j in perm])
    gw_p = np.asarray([widths[j] for j in perm], np.int64)
    pb = np.concatenate(([0], np.cumsum(gw_p))).astype(np.int64)
    cc_p = np.asarray([deg_fin[pb[i] : pb[i + 1]].max() for i in range(NG)])
    return node_fin, deg_fin, cc_p


def _prep_core(c, meta, src, dst, att_flat, entity_embed, cc_p):
    """Host-side packing for one core. Returns the per-core input map."""
    node_fin, deg_fin, _ = meta
    gw_p, nboff, sbs, na, nb, aoff, boff, sb_base, totf = _schedule(cc_p)
    gw_p = np.asarray(gw_p, np.int64)
    na_arr = np.asarray(na, np.int64)
    abase = np.empty(NG, np.int64)
    bbase = np.empty(NG, np.int64)
    for s, sb in enumerate(sbs):
        for i in sb:
            abase[i] = sb_base[s][0] + aoff[i]
            bbase[i] = sb_base[s][2] + boff[i]

    mask = (dst >= c * NPC) & (dst < (c + 1) * NPC)
    ld = (dst[mask] - c * NPC).astype(np.int64)
    e_src = src[mask]
    e_att = att_flat[mask]

    fr_of_node = np.empty(NPC, np.int64)
    real = node_fin >= 0
    fr_of_node[node_fin[real]] = np.nonzero(real)[0]
    er = fr_of_node[ld]                          # edge -> final dst rank

    order = np.argsort(er, kind="stable")
    er_s = er[order]
    starts_all = np.zeros(NPC_PAD + 1, np.int64)
    cnt = np.bincount(er_s, minlength=NPC_PAD)
    starts_all[1:] = np.cumsum(cnt)
    occ = np.arange(len(er_s)) - starts_all[er_s]

    pos = np.searchsorted(nboff, er_s, side="right") - 1
    i_in = er_s - nboff[pos]
    in_a = occ < na_arr[pos]
    cols = np.where(
        in_a,
        abase[pos] + occ * gw_p[pos] + i_in,
        bbase[pos] + (occ - na_arr[pos]) * gw_p[pos] + i_in)

    prod = (entity_embed[e_src[order]] * e_att[order, None]).astype(np.float16)
    arr = np.zeros((totf, D), np.float16)
    arr[cols] = prod
    msgs = np.ascontiguousarray(arr.T)           # [D, TOTF]

    ep = np.zeros((NPC_PAD, D), np.float16)
    ep[real] = entity_embed[c * NPC + node_fin[real]]
    embedT = np.ascontiguousarray(ep.T)          # [D, NPC_PAD]

    return dict(msgs=msgs, embedT=embedT)


def kernel(entity_embed, att, W1, b1, W2, b2, src, dst):
    from concourse.bass_utils import run_bass_kernel_spmd

    entity_embed = np.ascontiguousarray(np.asarray(entity_embed, dtype=np.float32))
    att_flat = np.asarray(att, dtype=np.float32).reshape(-1)
    W1h = np.asarray(W1, dtype=np.float16)
    W2h = np.asarray(W2, dtype=np.float16)
    b1c = np.asarray(b1, dtype=np.float32).reshape(D, 1)
    b2c = np.asarray(b2, dtype=np.float32).reshape(D, 1)
    src = np.asarray(src).astype(np.int64)
    dst = np.asarray(dst).astype(np.int64)

    metas = [_core_meta(c, dst) for c in range(NCORES)]
    cc_p = np.maximum(np.stack([m[2] for m in metas]).max(axis=0), 1)
    cc_p = cc_p.astype(np.int64)

    shared = dict(w1=W1h, w2=W2h, b1=b1c, b2=b2c)
    in_maps = []
    for c in range(NCORES):
        m = _prep_core(c, metas[c], src, dst, att_flat, entity_embed, cc_p)
        m.update(shared)
        in_maps.append(m)

    nc = _build(cc_p)
    res = run_bass_kernel_spmd(nc, in_maps, core_ids=list(range(NCORES)))

    out = np.empty((N_NODES, D), np.float32)
    for c in range(NCORES):
        o = res.results[c]["outT"]               # [128d, NPC_PAD] fp16
        o = o.T.astype(np.float32)               # [NPC_PAD, 128]
        node_fin = metas[c][0]
        real = node_fin >= 0
        blk = out[c * NPC : (c + 1) * NPC]
        blk[node_fin[real]] = o[real]
    return out
